# revision 13
# baseline (speedup 1.0000x reference)
"""Trainium2 Bass kernel for nn_BoundaryLoss (3D-Laplacian boundary loss).

reference semantics (fp32):
    probs = softmax(logits, axis=1)[:, 1:]                  # (B, C-1, D, H, W)
    tmask = one_hot(targets)[classes 1..C-1]                # (B, C-1, D, H, W)
    loss  = mean((|lap3(probs)| - |lap3(tmask)|)**2)        # lap3 = 6-neighbour
                                                            # Laplacian, zero pad

Distribution: pure data parallelism over H (256 rows -> 8 slices of 32 rows,
plus one halo row on each side).  Each core computes partial sums of
a^2, b^2 and |a*b| (a = lap(probs), b = lap(mask)); the host combines
sum((|a|-|b|)^2) = sum(a^2) + sum(b^2) - 2*sum(|a*b|) and divides by the
global element count.  This form needs no on-chip abs on the DVE (abs_max
is not a valid TRN2 ALU op) and keeps every per-element op on a fast engine.

On-core layout: SBUF partitions = (b, d) = 2*64 = 128, free dim = (h, w).
The stencil runs on the TensorEngine in 5 accumulating passes per 2-row
output region:
  - d+-1 and the -6 center via a block-tridiagonal stationary (T_D)
  - h+-1 via identity-stationary matmuls on rhs rows r+-1
  - w+-1 via identity-stationary matmuls with partial-width out/rhs columns
    (out[:, 1:W] += src[:, 0:W-1] etc.) -- implements the zero pad exactly.

All non-PE work is done in 8-row mega-groups (2048 elements per partition)
to amortize the per-instruction fixed overheads (~224 ScalarE / ~151 DVE
cycles).

Engine schedule (per core), phased for overlap:
  DVE:     class masks (is_equal), mask-lap PSUM->SBUF copies, sum(b^2)
           via tensor_tensor_reduce, reciprocal_approx_fast + cast,
           p = e*r mults, q = a*b mults
  ScalarE: exp x4, sum(a^2) (Square+accum off PSUM), sum(|a*b|) (Abs+accum)
  PE:      mask laps c1,c2 -> denominator accumulation -> mask lap c3 ->
           probs laps c1..c3 (keeps the PE stream dense so the HAM clock
           gate stays at 2.4 GHz)
"""

import numpy as np
import ml_dtypes

import concourse.bass as bass
import concourse.bacc as bacc
import concourse.tile as tile
from concourse import mybir
from concourse.bass_utils import run_bass_kernel_spmd

# Problem shape (hardcoded; harness contract)
B, C, D, H, W = 2, 4, 64, 256, 256
NCORES = 8
HS = H // NCORES        # 32 output rows per core
HL = HS + 2             # 34 input rows (1 halo row each side)
GROUP = 8               # output rows per PSUM tile (4 banks)
NG = HS // GROUP        # 4 lap mega-groups per class-tensor
NEG = -100.0            # pad value for classes 1..3 -> softmax prob ~ 0
NTOT = B * (C - 1) * D * H * W  # mean denominator

F32 = mybir.dt.float32
BF16 = mybir.dt.bfloat16
AX = mybir.AxisListType
OP = mybir.AluOpType
AF = mybir.ActivationFunctionType

NGT = 3 * NG  # (class, mega-group) pairs

# denominator groups over all HL rows: (row0, nrows)
DGROUPS = [(r0, min(GROUP, HL - r0)) for r0 in range(0, HL, GROUP)]


def _stationaries():
    """T_D: d-stencil (d+-1 within the same b, -6 on the diagonal) on the
    interleaved partition layout p = 2*d + b.  wI: identity.  Exact in bf16."""
    td = np.zeros((128, 128), dtype=np.float32)
    for p in range(128):
        td[p, p] = -6.0
        d, b = divmod(p, 2)
        if d > 0:
            td[p - 2, p] = 1.0
        if d < D - 1:
            td[p + 2, p] = 1.0
    ident = np.eye(128, dtype=np.float32)
    return (td.astype(ml_dtypes.bfloat16), ident.astype(ml_dtypes.bfloat16))


def _emit(tc):
    nc = tc.nc
    # host pre-interleaves to partition order p = 2*d + b, so every DMA is a
    # plain 2D full-partition transfer
    lg = nc.dram_tensor("logits", [C, 128, HL, W], BF16, kind="ExternalInput").ap()
    tg = nc.dram_tensor("targets", [128, HL, W], BF16, kind="ExternalInput").ap()
    wtd_d = nc.dram_tensor("wTD", [128, 128], BF16, kind="ExternalInput").ap()
    wi_d = nc.dram_tensor("wI", [128, 128], BF16, kind="ExternalInput").ap()
    out_d = nc.dram_tensor("out", [128, 3], F32, kind="ExternalOutput").ap()

    with (
        tc.tile_pool(name="singles", bufs=1) as singles,
        tc.tile_pool(name="mpool", bufs=2) as mpool,
        tc.tile_pool(name="bpool", bufs=12) as bpool,
        tc.tile_pool(name="rfpool", bufs=1) as rfpool,
        tc.tile_pool(name="qpool", bufs=1) as qpool,
        tc.tile_pool(name="spool", bufs=1) as spool,
        tc.tile_pool(name="vpool", bufs=1) as vpool,
        tc.tile_pool(name="psum", bufs=2, space="PSUM") as psum,
    ):
        # --- constants / persistent tiles ---
        wtd = singles.tile([128, 128], BF16, tag="wtd")
        wi = singles.tile([128, 128], BF16, tag="wi")
        # accumulator slots: [sum(a^2) x12 | sum(b^2) x12 | sum(|ab|) x12]
        slots = singles.tile([128, 3 * NGT], F32, tag="slots")
        res = singles.tile([128, 3], F32, tag="res")
        tgt = singles.tile([128, HL, W], BF16, tag="tgt")
        rball = singles.tile([128, HL * W], BF16, tag="rball")

        # DMAs: targets first (mask phase gates the PE start), then weights,
        # then logits.  Each tile is a fresh SBUF slot so no DMA carries a
        # sync wait (the DMA DIRECT2D pseudo-op supports at most one).
        nc.sync.dma_start(out=tgt, in_=tg)
        nc.sync.dma_start(out=wtd, in_=wtd_d)
        nc.sync.dma_start(out=wi, in_=wi_d)
        xl = []
        for ci in range(C):
            t = singles.tile([128, HL, W], BF16, tag=f"xl{ci}")
            nc.sync.dma_start(out=t, in_=lg[ci])
            xl.append(t)

        # --- DVE: class masks 1..2 (class 3 later, its buffer rotates) ---
        m = {}
        for ci in (1, 2):
            mt = mpool.tile([128, HL, W], BF16, tag="m")
            nc.vector.tensor_scalar(mt, tgt, float(ci), None, OP.is_equal)
            m[ci] = mt

        # --- ScalarE: exp of all 4 classes, in place (bf16) ---
        for ci in range(C):
            nc.scalar.activation(xl[ci], xl[ci], AF.Exp)
        e = xl  # after exp: e[ci] holds exp(logits_ci)

        def lap(pt, src, g):
            """Accumulate the 7-point Laplacian of src into psum tile pt for
            output rows r0..r0+GROUP-1 (absolute tile rows r0 = 1 + 8g).
            Zero w-pad comes from partial-width passes; h-pad from the halo
            rows; d-pad from the T_D stationary's missing off-diagonals."""
            r0 = 1 + g * GROUP
            js = range(0, GROUP, 2)
            for j in js:  # pass 1: d+-1 and -6 center
                nc.tensor.matmul(
                    out=pt[:, j:j + 2, :], lhsT=wtd,
                    rhs=src[:, r0 + j:r0 + j + 2, :],
                    start=True, stop=False)
            for j in js:  # pass 2: w-1  (out col w += src col w-1)
                nc.tensor.matmul(
                    out=pt[:, j:j + 2, 1:W], lhsT=wi,
                    rhs=src[:, r0 + j:r0 + j + 2, 0:W - 1],
                    start=False, stop=False)
            for j in js:  # pass 3: w+1
                nc.tensor.matmul(
                    out=pt[:, j:j + 2, 0:W - 1], lhsT=wi,
                    rhs=src[:, r0 + j:r0 + j + 2, 1:W],
                    start=False, stop=False)
            for j in js:  # pass 4: h-1
                nc.tensor.matmul(
                    out=pt[:, j:j + 2, :], lhsT=wi,
                    rhs=src[:, r0 + j - 1:r0 + j + 1, :],
                    start=False, stop=False)
            for j in js:  # pass 5: h+1 (closes the accumulation group)
                nc.tensor.matmul(
                    out=pt[:, j:j + 2, :], lhsT=wi,
                    rhs=src[:, r0 + j + 1:r0 + j + 3, :],
                    start=False, stop=True)

        bt = {}

        def mask_phase(ci):
            """PE: lap of mask ci, drained signed to bf16 (exact: small ints)
            by the DVE, which also accumulates sum(b^2) via TTR."""
            for g in range(NG):
                pt = psum.tile([128, GROUP, W], F32, tag="ps")
                lap(pt, m[ci], g)
                b = bpool.tile([128, GROUP, W], BF16, tag="b")
                nc.vector.tensor_copy(b, pt)
                bt[(ci, g)] = b
                idx = (ci - 1) * NG + g
                scr = vpool.tile([128, GROUP, W], BF16, tag="vscr")
                nc.scalar.activation(
                    scr, b, AF.Square,
                    accum_out=slots[:, NGT + idx:NGT + idx + 1])

        mask_phase(1)
        mask_phase(2)

        # --- PE: denominator accumulation; DVE: recip + cast per group ---
        for r0, nr in DGROUPS:
            st = psum.tile([128, GROUP, W], F32, tag="ps")
            for ci in range(C):
                for j in range(0, nr, 2):
                    nc.tensor.matmul(
                        out=st[:, j:j + 2, :], lhsT=wi,
                        rhs=e[ci][:, r0 + j:r0 + j + 2, :],
                        start=(ci == 0), stop=(ci == C - 1))
            sflat = st[:, 0:nr, :].rearrange("p h w -> p (h w)")
            rf = rfpool.tile([128, GROUP * W], F32, tag="rf")
            nc.vector.reciprocal_approx_fast(out=rf[:, 0:nr * W], in_=sflat)
            nc.vector.tensor_copy(rball[:, r0 * W:(r0 + nr) * W],
                                  rf[:, 0:nr * W])

        # class-3 mask now (rotates into mask-1's buffer, which the PE is
        # done reading)
        mt = mpool.tile([128, HL, W], BF16, tag="m")
        nc.vector.tensor_scalar(mt, tgt, 3.0, None, OP.is_equal)
        m[3] = mt

        # --- DVE: p = e * r, class-major so class 1 unblocks first ---
        for ci in range(1, C):
            ev = e[ci][:, :, :].rearrange("p h w -> p (h w)")
            nc.vector.tensor_tensor(out=ev, in0=ev, in1=rball, op=OP.mult)

        # mask lap class 3
        mask_phase(3)

        # --- probs laps + tail ---
        for ci in range(1, C):
            for g in range(NG):
                pp = psum.tile([128, GROUP, W], F32, tag="ps")
                lap(pp, e[ci], g)
                idx = (ci - 1) * NG + g
                # sum(a^2) straight off PSUM
                scr = spool.tile([128, GROUP, W], BF16, tag="scr")
                nc.scalar.activation(
                    scr, pp, AF.Square, accum_out=slots[:, idx:idx + 1])
                # q = a*b
                q = qpool.tile([128, GROUP, W], BF16, tag="q")
                nc.vector.tensor_tensor(
                    out=q, in0=pp, in1=bt[(ci, g)], op=OP.mult)
                # sum(|a*b|)
                scr2 = spool.tile([128, GROUP, W], BF16, tag="scr")
                nc.scalar.activation(
                    scr2, q, AF.Abs,
                    accum_out=slots[:, 2 * NGT + idx:2 * NGT + idx + 1])

        for r in range(3):
            nc.vector.reduce_sum(
                out=res[:, r:r + 1], in_=slots[:, r * NGT:(r + 1) * NGT],
                axis=AX.X)
        nc.sync.dma_start(out=out_d, in_=res)


def build_nc():
    nc = bacc.Bacc("TRN2", target_bir_lowering=False, debug=False)
    with tile.TileContext(nc) as tc:
        _emit(tc)
    nc.compile()
    return nc


_CACHE = {}


def _get_nc():
    if "nc" not in _CACHE:
        _CACHE["nc"] = build_nc()
    return _CACHE["nc"]


def make_in_maps(logits, targets):
    """Host-side marshaling: H-pad, slice per core, dtype-cast."""
    logits = np.asarray(logits, dtype=np.float32)
    targets = np.asarray(targets)
    # pad H by one row on each side: class0 logit 0, classes 1..3 -> NEG so
    # softmax probs vanish there (matches the reference's zero-padded conv on
    # probs); targets pad = class 0 -> masks vanish.
    lp = np.full((B, C, D, H + 2, W), 0.0, dtype=np.float32)
    lp[:, 1:, :, :, :] = NEG
    lp[:, :, :, 1:H + 1, :] = logits
    lp = lp.astype(ml_dtypes.bfloat16)
    tp = np.zeros((B, D, H + 2, W), dtype=np.float32)
    tp[:, :, 1:H + 1, :] = targets.astype(np.float32)
    tp = tp.astype(ml_dtypes.bfloat16)

    # interleave partitions: p = 2*d + b
    lp = lp.transpose(1, 2, 0, 3, 4).reshape(C, 2 * D, H + 2, W)
    tp = tp.transpose(1, 0, 2, 3).reshape(2 * D, H + 2, W)

    wtd, wi = _stationaries()
    in_maps = []
    for k in range(NCORES):
        h0 = k * HS
        in_maps.append({
            "logits": np.ascontiguousarray(lp[:, :, h0:h0 + HL, :]),
            "targets": np.ascontiguousarray(tp[:, h0:h0 + HL, :]),
            "wTD": wtd,
            "wI": wi,
        })
    return in_maps


def kernel(logits, targets):
    nc = _get_nc()
    in_maps = make_in_maps(logits, targets)
    results = run_bass_kernel_spmd(nc, in_maps, core_ids=list(range(NCORES)))
    # per-core out[:, 0] = sum(a^2), [:, 1] = sum(b^2), [:, 2] = sum(|a*b|)
    total = 0.0
    for r in results.results:
        o = np.asarray(r["out"], dtype=np.float64)
        total += o[:, 0].sum() + o[:, 1].sum() - 2.0 * o[:, 2].sum()
    return np.float32(total / NTOT)
